# revision 3
# baseline (speedup 1.0000x reference)
"""Trainium2 Bass kernel for nn_Attention_15771119911478 (RBF attention w/ RoPE).

Sharding: core h (of 8) computes head h for both batches (packed on partition
halves). Per-core output is the head's contribution to out @ Wo.T in [s, e]
layout, minus a per-row factor exp(-g*qn[s]) applied on the host. Host sums
the 8 per-core partials.

Host prep per head (cheap O(S*d^2)):
  qro = rope(q @ Wq_h.T).T          [64, S] per batch, bf16
  kro = 2g * rope(q @ Wk_h.T).T     [64, S] per batch, bf16
  vsb = (q @ W_vo) * exp(-g*kn)[:,None]  (kn bias folded in as a
                                     multiplicative factor; strip-blocked)
Device math per core:
  scs[t,s] = exp(kro[:,t].qro[:,s])              (bias-free exp)
  out2[s,e] = sum_t scs[t,s] * vsb[t,e]

Engine balance (cost-model driven): the exp over ~4.46M score elements is
split per strip region at a cut point: the left span (which contains the
causal diagonal block) runs on DVE as a one-op Schraudolph fast-exp
(scalar_tensor_tensor: i16(x*A16 + Bmask) bitcast to bf16; the f32 bias
tensor holds B16 off-diagonal and -1e9 in the masked triangle, which
saturates to -32768 = bf16 -0.0 — so causal masking costs nothing), and the
right span runs on ACT as exact Exp. PSUM slot machinery (2 x 4-bank slots,
sv parked in dead psum, per-strip evacs) is unchanged; evacs alternate
between DVE and ACT to balance.

All PSUM goes through ONE pool tag ([128, 2048] f32 = 4 banks, bufs=2) so
slot reuse is semaphore-based, never a pool-boundary drain. Slot layout is
always b0 in banks 0-1 (cols 0:1024), b1 in banks 2-3 (cols 1024:2048):
a matmul psum write starting at a non-bank-aligned column crashes the device,
and each bank only ever sees one tile_position stream.
"""
import os
import sys

sys.path.insert(0, "/opt/trn_rl_repo")

import numpy as np
import ml_dtypes

S = 2048
D = 64
H = 8
B = 2
N_CORES = 8
SCALE = 1.0 / 8.0  # 1/sqrt(64)
BF16 = ml_dtypes.bfloat16

# Schraudolph fast-exp constants for direct bf16(i16) output:
# i16 = rnne(x * 128/ln2 + (127*128 - C16)); bit pattern read as bf16.
A16 = 128.0 / np.log(2.0)
C16 = 7.0
B16 = 127.0 * 128.0 - C16
MASK_NEG = -1.0e9  # saturates the i16 convert to -32768 = bf16 -0.0

# Per-region DVE spans (left, diag-containing part of each strip region).
# A-part of strip j covers s in [128j, 1024); B-part of strip i covers
# s in [max(1024, 128i), 2048). Widths: wA(j) = 1024-128j, wB as in sB/wB.
WD_A = {0: 512, 1: 512, 2: 512, 3: 512, 4: 512, 5: 384, 6: 256, 7: 128}
WD_B = {0: 0, 1: 0, 2: 0, 3: 0, 4: 0, 5: 0, 6: 0, 7: 0,
        8: 512, 9: 512, 10: 512, 11: 512, 12: 512, 13: 384, 14: 256, 15: 128}
# Strips whose sv-psum evac runs on ACT (nc.scalar.copy) instead of DVE.
EVAC_ACT = {1, 3, 5, 7, 9, 11, 13, 15}

_PROG = None
LAST_RESULTS = None


def _build_program():
    import concourse.bass as bass
    import concourse.bacc as bacc
    import concourse.tile as tile
    from concourse import mybir

    f32 = mybir.dt.float32
    bf16 = mybir.dt.bfloat16
    i16 = mybir.dt.int16
    Exp = mybir.ActivationFunctionType.Exp
    MULT = mybir.AluOpType.mult
    ADD = mybir.AluOpType.add

    nc = bacc.Bacc(
        "TRN2",
        target_bir_lowering=False,
        debug=False,
        enable_asserts=False,
        num_devices=N_CORES,
    )

    def din(name, shape, dt):
        return nc.dram_tensor(name, shape, dt, kind="ExternalInput").ap()

    t_bm = din("bmask", [128, 1024], f32)  # Schraudolph bias w/ causal mask
    t_qro = din("qro", [128, S], bf16)
    t_kro = din("kro", [128, S], bf16)
    t_vsb = din("vsb", [128, 2 * 1024], bf16)  # vsb per batch, strip-blocked
    t_out = nc.dram_tensor("out", [128, S], f32, kind="ExternalOutput").ap()

    SCT_OFF = {12: 0, 13: 512, 14: 896, 15: 1152}

    # strip geometry
    def wA(j):
        return max(0, 1024 - 128 * j)

    def sB(j):
        return max(1024, 128 * j)

    def wB(j):
        return 2048 - sB(j)

    def sc_col(i, j, b):
        # column of s-block i (abs) in scs[j] for batch b
        if 128 * i < 1024:
            return b * wA(j) + 128 * (i - j)
        if j >= 12:
            return SCT_OFF[j] + b * 1280 + 128 * i - sB(j)
        return 2 * wA(j) + b * wB(j) + 128 * i - sB(j)

    with tile.TileContext(nc) as tc:
        with (
            tc.tile_pool(name="const", bufs=1) as const,
            tc.tile_pool(name="big", bufs=1) as big,
            tc.tile_pool(name="scp", bufs=1) as scp,
            tc.tile_pool(name="pp", bufs=2, space="PSUM") as pp,
        ):
            # ---- SBUF tiles ----
            bmask = const.tile([128, 1024], f32, tag="bmask")
            qro = big.tile([128, S], bf16, tag="qro")
            kro = big.tile([128, S], bf16, tag="kro")
            vsbt = big.tile([128, 2 * 1024], bf16, tag="vsbt")
            vsb = [vsbt[:, 0:1024], vsbt[:, 1024:2048]]
            outsb = big.tile([128, S], f32, tag="outsb")
            scs = {}
            for j in range(12):
                scs[j] = scp.tile(
                    [128, 2 * (2048 - 128 * j)], bf16, tag=f"sc_{j}", name=f"sc_{j}"
                )
            # strips 12-15 share one tile, per-batch regions packed
            sct = scp.tile([128, 2 * 1280], bf16, tag="sct", name="sct")
            for j in (12, 13, 14, 15):
                scs[j] = sct
            sct3 = sct.rearrange("p (b c) -> p b c", b=2)

            bm3 = bmask.rearrange("p (b c) -> p b c", b=2)  # [128, 2, 512]

            def slot():
                return pp.tile([128, 2048], f32, tag="slot", name="slot")

            # ---- input DMAs: first qk strips need kro/qro low cols; the
            # bmask is needed by the first DVE exp (~1.5us in)
            nc.sync.dma_start(kro[:, 512:1024], t_kro[:, 512:1024])
            nc.sync.dma_start(qro[:, 512:1024], t_qro[:, 512:1024])
            nc.sync.dma_start(kro[:, 0:512], t_kro[:, 0:512])
            nc.sync.dma_start(qro[:, 0:512], t_qro[:, 0:512])
            nc.sync.dma_start(bmask[:], t_bm[:])
            nc.sync.dma_start(qro[:, 1024:2048], t_qro[:, 1024:2048])
            nc.sync.dma_start(kro[:, 1024:2048], t_kro[:, 1024:2048])
            nc.sync.dma_start(vsbt[:], t_vsb[:])

            # preload ACT exp table from the first-arriving DMA chunk
            scratch = const.tile([128, 1], f32, tag="scratch")
            nc.scalar.activation(scratch[:], kro[:, 512:513], Exp)

            def qk_mms(dst, b, j, s0, s1, base=0):
                # qk matmuls for strip j, batch b, abs s-range [s0, s1) into
                # psum dst cols [b*1024+base ...); split at 512 bank boundaries
                rows = slice(64 * b, 64 * b + 64)
                tp = (0, 0) if b == 0 else (64, 0)
                off = 0
                while s0 + off < s1:
                    c = base + off
                    wc = min(512 - c % 512, s1 - s0 - off)
                    nc.tensor.matmul(
                        dst[:, b * 1024 + c : b * 1024 + c + wc],
                        kro[rows, j * 128 : j * 128 + 128],
                        qro[rows, s0 + off : s0 + off + wc],
                        start=True, stop=True, tile_position=tp,
                    )
                    off += wc

            def emit_exp(ps, j, col, w, wd, masked, pc=0, out3=None):
                # exp for strip j's region: psum cols [pc, pc+w) per batch
                # (b-stride 1024) -> scs[j] cols [col, col+2w) (b-stride w).
                # Left span [0, wd): DVE Schraudolph (mask fused via bmask
                # bias when `masked`). Right span [wd, w): ACT exact Exp.
                ps3 = ps.rearrange("p (b c) -> p b c", b=2)[:, :, pc : pc + w]
                if out3 is None:
                    out3 = scs[j][:, col : col + 2 * w].rearrange(
                        "p (b c) -> p b c", b=2
                    )
                if wd > 0:
                    o = out3[:, :, 0:wd].bitcast(i16)
                    if masked:
                        nc.vector.scalar_tensor_tensor(
                            o, ps3[:, :, 0:wd], A16, bm3[:, :, 0:wd], MULT, ADD
                        )
                    else:
                        nc.vector.tensor_scalar(
                            o, ps3[:, :, 0:wd], A16, B16, MULT, ADD
                        )
                if wd < w:
                    nc.scalar.activation(
                        out3[:, :, wd:w], ps3[:, :, wd:w], Exp
                    )

            def emit_A(j):
                ps = slot()
                for b in (0, 1):
                    qk_mms(ps, b, j, 128 * j, 1024)
                emit_exp(ps, j, 0, wA(j), WD_A[j], masked=True)

            bslots = {}

            def emit_qkB(i):
                ps = slot()
                bslots[i] = ps
                for b in (0, 1):
                    qk_mms(ps, b, i, sB(i), 2048)

            def emit_expB(i):
                emit_exp(bslots[i], i, 2 * wA(i), wB(i), WD_B[i], masked=(i >= 8))

            def sv_mms(ps, i, pc, js, start_j=0, stop_j=None):
                if stop_j is None:
                    stop_j = i
                for b in (0, 1):
                    for j in js:
                        nc.tensor.matmul(
                            ps[:, b * 1024 + pc : b * 1024 + pc + 64],
                            scs[j][:, sc_col(i, j, b) : sc_col(i, j, b) + 128],
                            vsb[b][:, j * 64 : j * 64 + 64],
                            start=(j == start_j), stop=(j == stop_j),
                        )

            def sv_evac(ps, i, pc):
                pout3 = ps.rearrange("p (b c) -> p b c", b=2)[:, :, pc : pc + 64]
                out3 = outsb[:, 128 * i : 128 * i + 128].rearrange(
                    "p (b c) -> p b c", b=2
                )
                if i in EVAC_ACT:
                    nc.scalar.copy(out3, pout3)
                else:
                    nc.vector.tensor_copy(out3, pout3)
                bslots.pop(i)

            def emit_sv(i):
                # out2 strip i accumulates in dead psum of slot i; for strips
                # with a dead tail (wB<1024) park there so the next slot
                # user's qk writes don't overlap the evac region
                pc = wB(i) if wB(i) <= 960 else 0
                sv_mms(bslots[i], i, pc, range(i + 1))
                sv_evac(bslots[i], i, pc)

            # ---- emission order: descending-width A (long exps last so B0's
            # qk hides under them) ----
            emit_A(7)
            emit_A(6)
            emit_A(5)
            emit_A(4)
            emit_A(3)
            emit_A(2)
            emit_A(1)
            emit_A(0)

            emit_qkB(0)
            emit_expB(0)
            for i in range(1, 12):
                emit_qkB(i)
                emit_sv(i - 1)
                emit_expB(i)
                if i % 4 == 0:
                    k = i // 4 - 1
                    nc.sync.dma_start(
                        t_out[:, 512 * k : 512 * k + 512],
                        outsb[:, 512 * k : 512 * k + 512],
                    )
            # strips 12-15: two strips per slot (wB <= 512); qk for the pair
            # lands before either exp
            pc_of = {12: 0, 13: 512, 14: 0, 15: 256}
            for a in (12, 14):
                ps = slot()
                for ii in (a, a + 1):
                    bslots[ii] = ps
                    for b in (0, 1):
                        qk_mms(ps, b, ii, sB(ii), 2048, base=pc_of[ii])
                if a == 12:
                    emit_sv(11)
                for ii in (a, a + 1):
                    out3 = sct3[:, :, SCT_OFF[ii] : SCT_OFF[ii] + wB(ii)]
                    emit_exp(
                        ps, ii, 0, wB(ii), WD_B[ii], masked=True,
                        pc=pc_of[ii], out3=out3,
                    )
                if a == 12:
                    nc.sync.dma_start(t_out[:, 1024:1536], outsb[:, 1024:1536])
            # tail: interleave sv chains across the two pair tiles so each
            # chain's evac hides under the other tile's sv matmuls
            ps12, ps14 = bslots[12], bslots[14]
            sv_mms(ps12, 12, 0, range(13))
            sv_evac(ps12, 12, 0)
            sv_mms(ps14, 14, 512, range(15))
            sv_evac(ps14, 14, 512)
            sv_mms(ps12, 13, 512, range(14))
            sv_evac(ps12, 13, 512)
            nc.sync.dma_start(t_out[:, 1536:1920], outsb[:, 1536:1920])
            sv_mms(ps14, 15, 576, range(16))
            sv_evac(ps14, 15, 576)
            nc.sync.dma_start(t_out[:, 1920:2048], outsb[:, 1920:2048])

    nc.compile()
    return nc


def _get_program():
    global _PROG
    if _PROG is None:
        _PROG = _build_program()
    return _PROG


def _rope_T(x):
    # interleaved RoPE on [S, 64], returns [64, S] f32
    f = np.arange(32, dtype=np.float64)
    freqs = 1.0 / (10000.0 ** (2 * f / 64))
    ang = np.arange(S, dtype=np.float64)[:, None] * freqs[None, :]
    c = np.cos(ang)
    s = np.sin(ang)
    x1, x2 = x[:, 0::2].astype(np.float64), x[:, 1::2].astype(np.float64)
    out = np.empty((S, 64), np.float64)
    out[:, 0::2] = x1 * c - x2 * s
    out[:, 1::2] = x1 * s + x2 * c
    return out.T.astype(np.float32)


def _prep_inputs(q, Wq, Wk, Wv, Wo, gamma):
    """Build the per-core in_maps (all host-side numpy)."""
    q = np.asarray(q, np.float32)
    Wq = np.asarray(Wq, np.float32)
    Wk = np.asarray(Wk, np.float32)
    Wv = np.asarray(Wv, np.float32)
    Wo = np.asarray(Wo, np.float32)
    gamma = np.asarray(gamma, np.float32)

    # Schraudolph bias tile [128, 2*512] f32: per-batch halves; triangle
    # (t > s masked -> -1e9) in cols 0:128 of each half, B16 elsewhere.
    bm = np.full((128, 1024), B16, np.float32)
    blocked = ~np.triu(np.ones((128, 128), bool))  # mask t > s (strictly)
    for h0 in (0, 512):
        bm[:, h0 : h0 + 128] = np.where(blocked, MASK_NEG, B16)

    in_maps = []
    qn_exp = np.zeros((B, H, S), np.float32)
    for h in range(H):
        g = float(gamma[h]) * SCALE
        Wq_h = Wq[h * 64 : (h + 1) * 64]
        Wk_h = Wk[h * 64 : (h + 1) * 64]
        Wv_h = Wv[h * 64 : (h + 1) * 64]
        Wo_h = Wo[:, h * 64 : (h + 1) * 64]  # [64(e), 64(d)]
        W_vo = Wv_h.T @ Wo_h.T  # [64(i), 64(e)] : q @ W_vo = vh @ Wo_h.T

        qro_b, kro_b, vsb_b = [], [], []
        for b in range(B):
            qh = q[b] @ Wq_h.T
            kh = q[b] @ Wk_h.T
            qro_b.append(_rope_T(qh))
            kro_b.append(_rope_T(kh) * (2.0 * g))
            kn = (kh * kh).sum(-1)
            w2 = (q[b] @ W_vo) * np.exp(-g * kn)[:, None]  # [S, 64]
            vsb_b.append(
                w2.reshape(16, 128, 64).transpose(1, 0, 2).reshape(128, 1024)
            )
            qn = (qh * qh).sum(-1)
            qn_exp[b, h] = np.exp(-g * qn)

        qro = np.concatenate(qro_b, 0).astype(BF16)  # [128, S]
        kro = np.concatenate(kro_b, 0).astype(BF16)
        vsb = np.concatenate(vsb_b, 1).astype(BF16)  # [128, 2*1024]

        in_maps.append(
            {
                "bmask": np.ascontiguousarray(bm),
                "qro": np.ascontiguousarray(qro),
                "kro": np.ascontiguousarray(kro),
                "vsb": np.ascontiguousarray(vsb),
            }
        )
    return in_maps, qn_exp


def kernel(q, Wq, Wk, Wv, Wo, gamma):
    global LAST_RESULTS
    from concourse import bass_utils

    nc = _get_program()
    in_maps, qn_exp = _prep_inputs(q, Wq, Wk, Wv, Wo, gamma)
    trace = bool(int(os.environ.get("KERNEL_TRACE", "0")))
    res = bass_utils.run_bass_kernel_spmd(
        nc, in_maps, core_ids=list(range(N_CORES)), trace=trace
    )
    LAST_RESULTS = res

    final = np.zeros((B, S, D), np.float32)
    for h in range(H):
        o = np.asarray(res.results[h]["out"], np.float32)  # [128, S]
        # col block i: [b0(64) | b1(64)] for s-strip i; row r = s offset
        o4 = o.reshape(128, 16, 2, 64)  # [r, i, b, e]
        for b in range(B):
            ob = o4[:, :, b, :].transpose(1, 0, 2).reshape(S, D)  # [s, e]
            final[b] += ob * qn_exp[b, h][:, None]
    return final


# revision 5
# speedup vs baseline: 1.0814x; 1.0814x over previous
"""Trainium2 Bass kernel for nn_Attention_15771119911478 (RBF attention w/ RoPE).

Sharding: core h (of 8) computes head h for both batches (packed on partition
halves). Per-core output is the head's contribution to out @ Wo.T in [s, e]
layout, minus a per-row factor exp(-g*qn[s]) applied on the host. Host sums
the 8 per-core partials.

Host prep per head (cheap O(S*d^2)):
  qro = rope(q @ Wq_h.T).T          [64, S] per batch, bf16
  kro = 2g * rope(q @ Wk_h.T).T     [64, S] per batch, bf16
  vsb = (q @ W_vo) * exp(-g*kn)[:,None]  (kn bias folded in as a
                                     multiplicative factor; strip-blocked)
Device math per core:
  scs[t,s] = exp(kro[:,t].qro[:,s])              (bias-free exp)
  out2[s,e] = sum_t scs[t,s] * vsb[t,e]

Engine balance (cost-model driven): the exp over ~4.46M score elements is
split per strip region between DVE (one-op Schraudolph fast-exp:
i16(x*A16 + bias) bitcast to bf16; for diag-bearing regions the f32 bias
tensor holds B16 off-diagonal and -1e9 in the causal triangle, which
saturates to -32768 = bf16 -0.0, so masking is free) and ACT (exact Exp).
Optionally a diag region runs entirely on ACT with the triangle zeroed
afterwards by a Pool (gpsimd) bf16 multiply in SBUF. Input DMAs are issued
from both SP and Pool queues to halve issue serialization.

All PSUM goes through ONE pool tag ([128, 2048] f32 = 4 banks, bufs=2) so
slot reuse is semaphore-based, never a pool-boundary drain. Slot layout is
always b0 in banks 0-1 (cols 0:1024), b1 in banks 2-3 (cols 1024:2048):
a matmul psum write starting at a non-bank-aligned column crashes the device,
and each bank only ever sees one tile_position stream.
"""
import os
import sys

sys.path.insert(0, "/opt/trn_rl_repo")

import numpy as np
import ml_dtypes

S = 2048
D = 64
H = 8
B = 2
N_CORES = 8
SCALE = 1.0 / 8.0  # 1/sqrt(64)
BF16 = ml_dtypes.bfloat16

# Schraudolph fast-exp constants for direct bf16(i16) output:
# i16 = rnne(x * 128/ln2 + (127*128 - C16)); bit pattern read as bf16.
A16 = 128.0 / np.log(2.0)
C16 = 7.0
B16 = 127.0 * 128.0 - C16
MASK_NEG = -1.0e9  # saturates the i16 convert to -32768 = bf16 -0.0

# ---- engine assignment tables (tuned against TimelineSim) ----
# Per A-part j (region s in [128j, 1024), width 1024-128j, diag at col 0):
#   wd: DVE Schraudolph span [0, wd) (fused mask; wd <= 512, >= 128 or 0)
#   pm: True -> whole region on ACT, diag triangle masked by Pool afterwards
WD_A = {0: 512, 1: 512, 2: 512, 3: 512, 4: 512, 5: 0, 6: 0, 7: 128}
PM_A = {0: False, 1: False, 2: False, 3: False, 4: False,
        5: True, 6: True, 7: False}
# Per B-part i (region s in [max(1024,128i), 2048); diag at col 0 for i>=8)
WD_B = {0: 512, 1: 512, 2: 512, 3: 512, 4: 512, 5: 512, 6: 512, 7: 512,
        8: 512, 9: 512, 10: 512, 11: 512, 12: 512, 13: 384, 14: 256, 15: 128}
PM_B = {i: False for i in range(16)}
# Strips whose sv-psum evac runs on ACT (nc.scalar.copy) instead of DVE.
EVAC_ACT = {1, 3, 5, 7, 9, 11, 13, 15}

_PROG = None
LAST_RESULTS = None


def _build_program():
    import concourse.bass as bass
    import concourse.bacc as bacc
    import concourse.tile as tile
    from concourse import mybir

    f32 = mybir.dt.float32
    bf16 = mybir.dt.bfloat16
    i16 = mybir.dt.int16
    Exp = mybir.ActivationFunctionType.Exp
    MULT = mybir.AluOpType.mult
    ADD = mybir.AluOpType.add

    nc = bacc.Bacc(
        "TRN2",
        target_bir_lowering=False,
        debug=False,
        enable_asserts=False,
        num_devices=N_CORES,
    )

    def din(name, shape, dt):
        return nc.dram_tensor(name, shape, dt, kind="ExternalInput").ap()

    t_bm = din("bmask", [128, 1024], f32)  # Schraudolph bias w/ causal mask
    t_mk = din("maskb", [128, 256], bf16)  # 0/1 triangle x2 batches (Pool)
    t_qro = din("qro", [128, S], bf16)
    t_kro = din("kro", [128, S], bf16)
    t_vsb = din("vsb", [128, 2 * 1024], bf16)  # vsb per batch, strip-blocked
    t_out = nc.dram_tensor("out", [128, S], f32, kind="ExternalOutput").ap()

    SCT_OFF = {12: 0, 13: 512, 14: 896, 15: 1152}

    # strip geometry
    def wA(j):
        return max(0, 1024 - 128 * j)

    def sB(j):
        return max(1024, 128 * j)

    def wB(j):
        return 2048 - sB(j)

    def sc_col(i, j, b):
        # column of s-block i (abs) in scs[j] for batch b
        if 128 * i < 1024:
            return b * wA(j) + 128 * (i - j)
        if j >= 12:
            return SCT_OFF[j] + b * 1280 + 128 * i - sB(j)
        return 2 * wA(j) + b * wB(j) + 128 * i - sB(j)

    with tile.TileContext(nc) as tc:
        with (
            tc.tile_pool(name="const", bufs=1) as const,
            tc.tile_pool(name="big", bufs=1) as big,
            tc.tile_pool(name="scp", bufs=1) as scp,
            tc.tile_pool(name="pp", bufs=2, space="PSUM") as pp,
        ):
            # ---- SBUF tiles ----
            bmask = const.tile([128, 1024], f32, tag="bmask")
            maskb = const.tile([128, 256], bf16, tag="maskb")
            mk3 = maskb.rearrange("p (b c) -> p b c", b=2)
            qro = big.tile([128, S], bf16, tag="qro")
            kro = big.tile([128, S], bf16, tag="kro")
            vsbt = big.tile([128, 2 * 1024], bf16, tag="vsbt")
            vsb = [vsbt[:, 0:1024], vsbt[:, 1024:2048]]
            outsb = big.tile([128, S], f32, tag="outsb")
            scs = {}
            for j in range(12):
                scs[j] = scp.tile(
                    [128, 2 * (2048 - 128 * j)], bf16, tag=f"sc_{j}", name=f"sc_{j}"
                )
            # strips 12-15 share one tile, per-batch regions packed
            sct = scp.tile([128, 2 * 1280], bf16, tag="sct", name="sct")
            for j in (12, 13, 14, 15):
                scs[j] = sct
            sct3 = sct.rearrange("p (b c) -> p b c", b=2)

            bm3 = bmask.rearrange("p (b c) -> p b c", b=2)  # [128, 2, 512]

            def slot():
                return pp.tile([128, 2048], f32, tag="slot", name="slot")

            # ---- input DMAs: SP queue takes the critical first chunks,
            # Pool (swdge) takes the rest so issue overhead parallelizes
            nc.sync.dma_start(kro[:, 512:1024], t_kro[:, 512:1024])
            nc.sync.dma_start(qro[:, 512:1024], t_qro[:, 512:1024])
            nc.sync.dma_start(kro[:, 0:512], t_kro[:, 0:512])
            nc.sync.dma_start(qro[:, 0:512], t_qro[:, 0:512])
            nc.gpsimd.dma_start(bmask[:], t_bm[:])
            nc.gpsimd.dma_start(maskb[:], t_mk[:])
            nc.gpsimd.dma_start(qro[:, 1024:2048], t_qro[:, 1024:2048])
            nc.gpsimd.dma_start(kro[:, 1024:2048], t_kro[:, 1024:2048])
            nc.gpsimd.dma_start(vsbt[:], t_vsb[:])

            # preload ACT exp table from the first-arriving DMA chunk
            scratch = const.tile([128, 1], f32, tag="scratch")
            nc.scalar.activation(scratch[:], kro[:, 512:513], Exp)

            def qk_mms(dst, b, j, s0, s1, base=0):
                # qk matmuls for strip j, batch b, abs s-range [s0, s1) into
                # psum dst cols [b*1024+base ...); split at 512 bank boundaries
                rows = slice(64 * b, 64 * b + 64)
                tp = (0, 0) if b == 0 else (64, 0)
                off = 0
                while s0 + off < s1:
                    c = base + off
                    wc = min(512 - c % 512, s1 - s0 - off)
                    nc.tensor.matmul(
                        dst[:, b * 1024 + c : b * 1024 + c + wc],
                        kro[rows, j * 128 : j * 128 + 128],
                        qro[rows, s0 + off : s0 + off + wc],
                        start=True, stop=True, tile_position=tp,
                    )
                    off += wc

            def emit_exp(ps, j, col, w, wd, masked, pool_mask=False,
                         pc=0, out3=None):
                # exp for strip j's region: psum cols [pc, pc+w) per batch
                # (b-stride 1024) -> scs[j] cols [col, col+2w) (b-stride w).
                # Left span [0, wd): DVE Schraudolph (mask fused via bmask
                # bias when `masked`). Right span [wd, w): ACT exact Exp.
                # pool_mask: wd must be 0; Pool zeroes the diag triangle in
                # SBUF after the ACT exp.
                ps3 = ps.rearrange("p (b c) -> p b c", b=2)[:, :, pc : pc + w]
                if out3 is None:
                    out3 = scs[j][:, col : col + 2 * w].rearrange(
                        "p (b c) -> p b c", b=2
                    )
                if wd > 0:
                    o = out3[:, :, 0:wd].bitcast(i16)
                    if masked:
                        nc.vector.scalar_tensor_tensor(
                            o, ps3[:, :, 0:wd], A16, bm3[:, :, 0:wd], MULT, ADD
                        )
                    else:
                        nc.vector.tensor_scalar(
                            o, ps3[:, :, 0:wd], A16, B16, MULT, ADD
                        )
                if wd < w:
                    nc.scalar.activation(
                        out3[:, :, wd:w], ps3[:, :, wd:w], Exp
                    )
                if pool_mask:
                    nc.gpsimd.tensor_mul(
                        out3[:, :, 0:128], out3[:, :, 0:128], mk3
                    )

            def emit_A(j):
                ps = slot()
                for b in (0, 1):
                    qk_mms(ps, b, j, 128 * j, 1024)
                emit_exp(ps, j, 0, wA(j), WD_A[j], masked=True,
                         pool_mask=PM_A[j])

            bslots = {}

            def emit_qkB(i):
                ps = slot()
                bslots[i] = ps
                for b in (0, 1):
                    qk_mms(ps, b, i, sB(i), 2048)

            def emit_expB(i):
                emit_exp(bslots[i], i, 2 * wA(i), wB(i), WD_B[i],
                         masked=(i >= 8), pool_mask=PM_B[i])

            def sv_mms(ps, i, pc, js, start_j=0, stop_j=None):
                if stop_j is None:
                    stop_j = i
                for b in (0, 1):
                    for j in js:
                        nc.tensor.matmul(
                            ps[:, b * 1024 + pc : b * 1024 + pc + 64],
                            scs[j][:, sc_col(i, j, b) : sc_col(i, j, b) + 128],
                            vsb[b][:, j * 64 : j * 64 + 64],
                            start=(j == start_j), stop=(j == stop_j),
                        )

            def sv_evac(ps, i, pc):
                pout3 = ps.rearrange("p (b c) -> p b c", b=2)[:, :, pc : pc + 64]
                out3 = outsb[:, 128 * i : 128 * i + 128].rearrange(
                    "p (b c) -> p b c", b=2
                )
                if i in EVAC_ACT:
                    nc.scalar.copy(out3, pout3)
                else:
                    nc.vector.tensor_copy(out3, pout3)
                bslots.pop(i)

            def emit_sv(i):
                # out2 strip i accumulates in dead psum of slot i; for strips
                # with a dead tail (wB<1024) park there so the next slot
                # user's qk writes don't overlap the evac region
                pc = wB(i) if wB(i) <= 960 else 0
                sv_mms(bslots[i], i, pc, range(i + 1))
                sv_evac(bslots[i], i, pc)

            # ---- emission order: descending-width A ----
            emit_A(7)
            emit_A(6)
            emit_A(5)
            emit_A(4)
            emit_A(3)
            emit_A(2)
            emit_A(1)
            emit_A(0)

            emit_qkB(0)
            emit_expB(0)
            for i in range(1, 12):
                emit_qkB(i)
                emit_sv(i - 1)
                emit_expB(i)
                if i % 4 == 0:
                    k = i // 4 - 1
                    nc.gpsimd.dma_start(
                        t_out[:, 512 * k : 512 * k + 512],
                        outsb[:, 512 * k : 512 * k + 512],
                    )
            # strips 12-15: two strips per slot (wB <= 512); qk for the pair
            # lands before either exp
            pc_of = {12: 0, 13: 512, 14: 0, 15: 256}
            for a in (12, 14):
                ps = slot()
                for ii in (a, a + 1):
                    bslots[ii] = ps
                    for b in (0, 1):
                        qk_mms(ps, b, ii, sB(ii), 2048, base=pc_of[ii])
                if a == 12:
                    emit_sv(11)
                for ii in (a, a + 1):
                    out3 = sct3[:, :, SCT_OFF[ii] : SCT_OFF[ii] + wB(ii)]
                    emit_exp(
                        ps, ii, 0, wB(ii), WD_B[ii], masked=True,
                        pool_mask=PM_B[ii], pc=pc_of[ii], out3=out3,
                    )
                if a == 12:
                    nc.gpsimd.dma_start(t_out[:, 1024:1536], outsb[:, 1024:1536])
            # tail: interleave sv chains across the two pair tiles so each
            # chain's evac hides under the other tile's sv matmuls
            ps12, ps14 = bslots[12], bslots[14]
            sv_mms(ps12, 12, 0, range(13))
            sv_evac(ps12, 12, 0)
            sv_mms(ps14, 14, 512, range(15))
            sv_evac(ps14, 14, 512)
            sv_mms(ps12, 13, 512, range(14))
            sv_evac(ps12, 13, 512)
            nc.sync.dma_start(t_out[:, 1536:1920], outsb[:, 1536:1920])
            sv_mms(ps14, 15, 576, range(16))
            sv_evac(ps14, 15, 576)
            nc.sync.dma_start(t_out[:, 1920:2048], outsb[:, 1920:2048])

    nc.compile()
    return nc


def _get_program():
    global _PROG
    if _PROG is None:
        _PROG = _build_program()
    return _PROG


def _rope_T(x):
    # interleaved RoPE on [S, 64], returns [64, S] f32
    f = np.arange(32, dtype=np.float64)
    freqs = 1.0 / (10000.0 ** (2 * f / 64))
    ang = np.arange(S, dtype=np.float64)[:, None] * freqs[None, :]
    c = np.cos(ang)
    s = np.sin(ang)
    x1, x2 = x[:, 0::2].astype(np.float64), x[:, 1::2].astype(np.float64)
    out = np.empty((S, 64), np.float64)
    out[:, 0::2] = x1 * c - x2 * s
    out[:, 1::2] = x1 * s + x2 * c
    return out.T.astype(np.float32)


def _prep_inputs(q, Wq, Wk, Wv, Wo, gamma):
    """Build the per-core in_maps (all host-side numpy)."""
    q = np.asarray(q, np.float32)
    Wq = np.asarray(Wq, np.float32)
    Wk = np.asarray(Wk, np.float32)
    Wv = np.asarray(Wv, np.float32)
    Wo = np.asarray(Wo, np.float32)
    gamma = np.asarray(gamma, np.float32)

    # Schraudolph bias tile [128, 2*512] f32: per-batch halves; triangle
    # (t > s masked -> -1e9) in cols 0:128 of each half, B16 elsewhere.
    bm = np.full((128, 1024), B16, np.float32)
    blocked = ~np.triu(np.ones((128, 128), bool))  # mask t > s (strictly)
    for h0 in (0, 512):
        bm[:, h0 : h0 + 128] = np.where(blocked, MASK_NEG, B16)
    mk = np.tile(np.triu(np.ones((128, 128), np.float32)), (1, 2)).astype(BF16)

    in_maps = []
    qn_exp = np.zeros((B, H, S), np.float32)
    for h in range(H):
        g = float(gamma[h]) * SCALE
        Wq_h = Wq[h * 64 : (h + 1) * 64]
        Wk_h = Wk[h * 64 : (h + 1) * 64]
        Wv_h = Wv[h * 64 : (h + 1) * 64]
        Wo_h = Wo[:, h * 64 : (h + 1) * 64]  # [64(e), 64(d)]
        W_vo = Wv_h.T @ Wo_h.T  # [64(i), 64(e)] : q @ W_vo = vh @ Wo_h.T

        qro_b, kro_b, vsb_b = [], [], []
        for b in range(B):
            qh = q[b] @ Wq_h.T
            kh = q[b] @ Wk_h.T
            qro_b.append(_rope_T(qh))
            kro_b.append(_rope_T(kh) * (2.0 * g))
            kn = (kh * kh).sum(-1)
            w2 = (q[b] @ W_vo) * np.exp(-g * kn)[:, None]  # [S, 64]
            vsb_b.append(
                w2.reshape(16, 128, 64).transpose(1, 0, 2).reshape(128, 1024)
            )
            qn = (qh * qh).sum(-1)
            qn_exp[b, h] = np.exp(-g * qn)

        qro = np.concatenate(qro_b, 0).astype(BF16)  # [128, S]
        kro = np.concatenate(kro_b, 0).astype(BF16)
        vsb = np.concatenate(vsb_b, 1).astype(BF16)  # [128, 2*1024]

        in_maps.append(
            {
                "bmask": np.ascontiguousarray(bm),
                "maskb": np.ascontiguousarray(mk),
                "qro": np.ascontiguousarray(qro),
                "kro": np.ascontiguousarray(kro),
                "vsb": np.ascontiguousarray(vsb),
            }
        )
    return in_maps, qn_exp


def kernel(q, Wq, Wk, Wv, Wo, gamma):
    global LAST_RESULTS
    from concourse import bass_utils

    nc = _get_program()
    in_maps, qn_exp = _prep_inputs(q, Wq, Wk, Wv, Wo, gamma)
    trace = bool(int(os.environ.get("KERNEL_TRACE", "0")))
    res = bass_utils.run_bass_kernel_spmd(
        nc, in_maps, core_ids=list(range(N_CORES)), trace=trace
    )
    LAST_RESULTS = res

    final = np.zeros((B, S, D), np.float32)
    for h in range(H):
        o = np.asarray(res.results[h]["out"], np.float32)  # [128, S]
        # col block i: [b0(64) | b1(64)] for s-strip i; row r = s offset
        o4 = o.reshape(128, 16, 2, 64)  # [r, i, b, e]
        for b in range(B):
            ob = o4[:, :, b, :].transpose(1, 0, 2).reshape(S, D)  # [s, e]
            final[b] += ob * qn_exp[b, h][:, None]
    return final


# revision 8
# speedup vs baseline: 1.2762x; 1.1801x over previous
"""Trainium2 Bass kernel for nn_Attention_15771119911478 (RBF attention w/ RoPE).

Sharding: core h (of 8) computes head h for both batches (packed on partition
halves). Per-core output is the head's contribution to out @ Wo.T in [s, e]
layout, minus a per-row factor exp(-g*qn[s]) applied on the host. Host sums
the 8 per-core partials.

Host prep per head (cheap O(S*d^2)):
  qro = rope(q @ Wq_h.T).T          [64, S] per batch, bf16
  kro = 2g * rope(q @ Wk_h.T).T     [64, S] per batch, bf16
  vsb = (q @ W_vo) * exp(-g*kn)[:,None]  (kn bias folded in as a
                                     multiplicative factor)
Device math per core:
  scs[t,s] = exp(kro[:,t].qro[:,s])              (bias-free exp)
  out2[s,e] = sum_t scs[t,s] * vsb[t,e]

Structure (cost-model driven): work is chunked at <=512 score columns per
batch. Chunk (j, c) = strip j (128 t-rows), s in [128j+512c, ...). Chunks
are emitted in a diagonal wavefront (sorted by s-extent), so qro/kro DMA
chunks, qk matmuls, exps, sv matmuls and evacs all pipeline in one pass.
PSUM: 3 x [128, 1024] f32 qk slots (2 banks each, b-stride 512) + 2 x
[128, 512] f32 sv-output tiles (1 bank each; 4 strips' sv outputs per tile,
one merged evac per 4 strips). sv_i is emitted as soon as the wave covers
s-block i for all strips j <= i.

The exp over ~4.46M score elements is split per chunk between DVE (one-op
Schraudolph fast-exp: i16(x*A16 + bias) bitcast to bf16; for the leading
diag chunk of each strip the f32 bias tensor holds B16 off-diagonal and
-1e9 in the causal triangle, which saturates to -32768 = bf16 -0.0, so
masking is free) and ACT (exact Exp), greedily balancing modeled engine
time. Input DMAs are split between the SP and Pool (swdge) queues.
"""
import os
import sys

sys.path.insert(0, "/opt/trn_rl_repo")

import numpy as np
import ml_dtypes

S = 2048
D = 64
H = 8
B = 2
N_CORES = 8
SCALE = 1.0 / 8.0  # 1/sqrt(64)
BF16 = ml_dtypes.bfloat16

# Schraudolph fast-exp constants for direct bf16(i16) output:
# i16 = rnne(x * 128/ln2 + (127*128 - C16)); bit pattern read as bf16.
A16 = 128.0 / np.log(2.0)
C16 = 7.0
B16 = 127.0 * 128.0 - C16
MASK_NEG = -1.0e9  # saturates the i16 convert to -32768 = bf16 -0.0

_PROG = None
LAST_RESULTS = None


def _chunks(j):
    """Chunk widths for strip j (s range [128j, 2048), <=512 per chunk)."""
    W = 2048 - 128 * j
    out = []
    while W > 0:
        out.append(min(512, W))
        W -= 512
    return out


def _wave_order():
    """(j, c) sorted by chunk s-extent, then strip."""
    items = []
    for j in range(16):
        for c, cw in enumerate(_chunks(j)):
            s_end = 128 * j + 512 * c + cw
            items.append((s_end, j, c, cw))
    items.sort()
    return items


def _assign_engines():
    """Greedy per-chunk engine split, balancing modeled ACT/DVE time.

    Returns (plan, evac_eng):
      plan[(j, c)] = wd  (DVE Schraudolph span [0, wd); ACT Exp [wd, cw)).
      For c == 0 the DVE span is causally masked via the bmask bias.
      evac_eng[k] in {"D", "A"} for the 4 merged sv evacs.
    """
    # modeled per-op costs (ns)
    dve_el, act_el = 1.0417, 0.8333
    dve_op, act_op = 195.0, 242.0
    load = {"D": 0.0, "A": 1283.0}  # ACT starts with the exp table load
    plan = {}
    evac_eng = {}
    sv_done = 0
    for s_end, j, c, cw in _wave_order():
        if c == 0:
            # leading chunk: DVE span must cover the diag (>=128) and is
            # masked. Before the full bmask has arrived (~wave 1280) only
            # wd=128 is safe (the mini bmask DMA covers the triangle).
            opts = [128] if s_end < 1280 else [128, 256, 384, min(512, cw)]
            opts = sorted({min(w, cw) for w in opts})
        else:
            opts = [0, cw]  # whole chunk to one engine
        best, bw = None, None
        for wd in opts:
            d = load["D"] + (2 * wd * dve_el + dve_op if wd > 0 else 0.0)
            a = load["A"] + (2 * (cw - wd) * act_el + act_op if wd < cw else 0.0)
            m = max(d, a)
            if best is None or m < best:
                best, bw = m, wd
        wd = bw
        plan[(j, c)] = wd
        if wd > 0:
            load["D"] += 2 * wd * dve_el + dve_op
        if wd < cw:
            load["A"] += 2 * (cw - wd) * act_el + act_op
        # merged evacs become ready as sv waves complete; model them when
        # the wave passes s = 128*(4k+3)+512
        while sv_done < 4 and s_end >= 128 * (4 * sv_done + 3) + 512:
            if load["D"] + 728 <= load["A"] + 669:
                evac_eng[sv_done] = "D"
                load["D"] += 728
            else:
                evac_eng[sv_done] = "A"
                load["A"] += 669
            sv_done += 1
    while sv_done < 4:
        evac_eng[sv_done] = "D" if load["D"] <= load["A"] else "A"
        sv_done += 1
    return plan, evac_eng


def _build_program():
    import concourse.bass as bass
    import concourse.bacc as bacc
    import concourse.tile as tile
    from concourse import mybir

    f32 = mybir.dt.float32
    bf16 = mybir.dt.bfloat16
    i16 = mybir.dt.int16
    Exp = mybir.ActivationFunctionType.Exp
    MULT = mybir.AluOpType.mult
    ADD = mybir.AluOpType.add

    plan, evac_eng = _assign_engines()

    nc = bacc.Bacc(
        "TRN2",
        target_bir_lowering=False,
        debug=False,
        enable_asserts=False,
        num_devices=N_CORES,
    )

    def din(name, shape, dt):
        return nc.dram_tensor(name, shape, dt, kind="ExternalInput").ap()

    t_bm = din("bmask", [128, 1024], f32)  # Schraudolph bias w/ causal mask
    t_qro = din("qro", [128, S], bf16)
    t_kro = din("kro", [128, S], bf16)
    t_vsb = din("vsb", [128, 2 * 1024], bf16)  # vsb per batch
    t_out = nc.dram_tensor("out", [128, S], f32, kind="ExternalOutput").ap()

    def Wj(j):
        return 2048 - 128 * j

    with tile.TileContext(nc) as tc:
        with (
            tc.tile_pool(name="const", bufs=1) as const,
            tc.tile_pool(name="big", bufs=1) as big,
            tc.tile_pool(name="scp", bufs=1) as scp,
            tc.tile_pool(name="pp", bufs=3, space="PSUM") as pp,
            tc.tile_pool(name="svp", bufs=2, space="PSUM") as svp,
        ):
            # ---- SBUF tiles ----
            bmask = const.tile([128, 1024], f32, tag="bmask")
            bm3 = bmask.rearrange("p (b c) -> p b c", b=2)  # [128, 2, 512]
            qro = big.tile([128, S], bf16, tag="qro")
            kro = big.tile([128, S], bf16, tag="kro")
            vsbt = big.tile([128, 2 * 1024], bf16, tag="vsbt")
            vsb = [vsbt[:, 0:1024], vsbt[:, 1024:2048]]
            outsb = big.tile([128, S], f32, tag="outsb")
            scs, scs3 = {}, {}
            for j in range(16):
                scs[j] = scp.tile(
                    [128, 2 * Wj(j)], bf16, tag=f"sc_{j}", name=f"sc_{j}"
                )
                scs3[j] = scs[j].rearrange("p (b c) -> p b c", b=2)

            def slot():
                return pp.tile([128, 1024], f32, tag="slot", name="slot")

            def svslot():
                return svp.tile([128, 512], f32, tag="svg", name="svg")

            # ---- input DMAs. SP: critical low chunks + bmask triangle
            # minis; Pool (swdge): the rest, in need order.
            nc.sync.dma_start(kro[:, 0:512], t_kro[:, 0:512])
            nc.sync.dma_start(qro[:, 0:512], t_qro[:, 0:512])
            nc.sync.dma_start(bmask[:, 0:128], t_bm[:, 0:128])
            nc.sync.dma_start(bmask[:, 512:640], t_bm[:, 512:640])
            nc.sync.dma_start(qro[:, 512:1024], t_qro[:, 512:1024])
            nc.sync.dma_start(kro[:, 512:1024], t_kro[:, 512:1024])
            nc.gpsimd.dma_start(bmask[:, 128:512], t_bm[:, 128:512])
            nc.gpsimd.dma_start(bmask[:, 640:1024], t_bm[:, 640:1024])
            nc.gpsimd.dma_start(qro[:, 1024:1536], t_qro[:, 1024:1536])
            nc.gpsimd.dma_start(vsbt[:, 0:512], t_vsb[:, 0:512])
            nc.gpsimd.dma_start(vsbt[:, 1024:1536], t_vsb[:, 1024:1536])
            nc.gpsimd.dma_start(qro[:, 1536:2048], t_qro[:, 1536:2048])
            nc.gpsimd.dma_start(kro[:, 1024:2048], t_kro[:, 1024:2048])
            nc.gpsimd.dma_start(vsbt[:, 512:1024], t_vsb[:, 512:1024])
            nc.gpsimd.dma_start(vsbt[:, 1536:2048], t_vsb[:, 1536:2048])

            # preload ACT exp table from the first-arriving DMA chunk
            scratch = const.tile([128, 1], f32, tag="scratch")
            nc.scalar.activation(scratch[:], kro[:, 0:1], Exp)

            def emit_chunk(j, c, cw):
                s0 = 128 * j + 512 * c
                ps = slot()
                for b in (0, 1):
                    rows = slice(64 * b, 64 * b + 64)
                    tp = (0, 0) if b == 0 else (64, 0)
                    nc.tensor.matmul(
                        ps[:, b * 512 : b * 512 + cw],
                        kro[rows, j * 128 : j * 128 + 128],
                        qro[rows, s0 : s0 + cw],
                        start=True, stop=True, tile_position=tp,
                    )
                ps3 = ps.rearrange("p (b c) -> p b c", b=2)[:, :, 0:cw]
                out3 = scs3[j][:, :, 512 * c : 512 * c + cw]
                wd = plan[(j, c)]
                if wd > 0:
                    o = out3[:, :, 0:wd].bitcast(i16)
                    if c == 0:
                        nc.vector.scalar_tensor_tensor(
                            o, ps3[:, :, 0:wd], A16, bm3[:, :, 0:wd], MULT, ADD
                        )
                    else:
                        nc.vector.tensor_scalar(
                            o, ps3[:, :, 0:wd], A16, B16, MULT, ADD
                        )
                if wd < cw:
                    nc.scalar.activation(out3[:, :, wd:cw], ps3[:, :, wd:cw], Exp)

            svg = {}

            def emit_sv(i):
                k = i // 4
                if i % 4 == 0:
                    svg[k] = svslot()
                ps = svg[k]
                for b in (0, 1):
                    col = (i % 4) * 128 + b * 64
                    for j in range(i + 1):
                        nc.tensor.matmul(
                            ps[:, col : col + 64],
                            scs[j][:, b * Wj(j) + 128 * (i - j) :
                                   b * Wj(j) + 128 * (i - j) + 128],
                            vsb[b][:, j * 64 : j * 64 + 64],
                            start=(j == 0), stop=(j == i),
                        )

            def emit_evac(k):
                if evac_eng[k] == "A":
                    nc.scalar.copy(
                        outsb[:, 512 * k : 512 * k + 512], svg[k][:, 0:512]
                    )
                else:
                    nc.vector.tensor_copy(
                        outsb[:, 512 * k : 512 * k + 512], svg[k][:, 0:512]
                    )
                nc.gpsimd.dma_start(
                    t_out[:, 512 * k : 512 * k + 512],
                    outsb[:, 512 * k : 512 * k + 512],
                )
                svg.pop(k)

            # ---- diagonal wavefront ----
            import itertools

            sv_next = 0
            for s_end, group in itertools.groupby(
                _wave_order(), key=lambda x: x[0]
            ):
                for _, j, c, cw in group:
                    emit_chunk(j, c, cw)
                while sv_next < 16 and s_end >= 128 * sv_next + 512:
                    emit_sv(sv_next)
                    if sv_next % 4 == 3:
                        emit_evac(sv_next // 4)
                    sv_next += 1
            while sv_next < 16:
                emit_sv(sv_next)
                if sv_next % 4 == 3:
                    emit_evac(sv_next // 4)
                sv_next += 1

    nc.compile()
    return nc


def _get_program():
    global _PROG
    if _PROG is None:
        _PROG = _build_program()
    return _PROG


def _rope_T(x):
    # interleaved RoPE on [S, 64], returns [64, S] f32
    f = np.arange(32, dtype=np.float64)
    freqs = 1.0 / (10000.0 ** (2 * f / 64))
    ang = np.arange(S, dtype=np.float64)[:, None] * freqs[None, :]
    c = np.cos(ang)
    s = np.sin(ang)
    x1, x2 = x[:, 0::2].astype(np.float64), x[:, 1::2].astype(np.float64)
    out = np.empty((S, 64), np.float64)
    out[:, 0::2] = x1 * c - x2 * s
    out[:, 1::2] = x1 * s + x2 * c
    return out.T.astype(np.float32)


def _prep_inputs(q, Wq, Wk, Wv, Wo, gamma):
    """Build the per-core in_maps (all host-side numpy)."""
    q = np.asarray(q, np.float32)
    Wq = np.asarray(Wq, np.float32)
    Wk = np.asarray(Wk, np.float32)
    Wv = np.asarray(Wv, np.float32)
    Wo = np.asarray(Wo, np.float32)
    gamma = np.asarray(gamma, np.float32)

    # Schraudolph bias tile [128, 2*512] f32: per-batch halves; triangle
    # (t > s masked -> -1e9) in cols 0:128 of each half, B16 elsewhere.
    bm = np.full((128, 1024), B16, np.float32)
    blocked = ~np.triu(np.ones((128, 128), bool))  # mask t > s (strictly)
    for h0 in (0, 512):
        bm[:, h0 : h0 + 128] = np.where(blocked, MASK_NEG, B16)

    in_maps = []
    qn_exp = np.zeros((B, H, S), np.float32)
    for h in range(H):
        g = float(gamma[h]) * SCALE
        Wq_h = Wq[h * 64 : (h + 1) * 64]
        Wk_h = Wk[h * 64 : (h + 1) * 64]
        Wv_h = Wv[h * 64 : (h + 1) * 64]
        Wo_h = Wo[:, h * 64 : (h + 1) * 64]  # [64(e), 64(d)]
        W_vo = Wv_h.T @ Wo_h.T  # [64(i), 64(e)] : q @ W_vo = vh @ Wo_h.T

        qro_b, kro_b, vsb_b = [], [], []
        for b in range(B):
            qh = q[b] @ Wq_h.T
            kh = q[b] @ Wk_h.T
            qro_b.append(_rope_T(qh))
            kro_b.append(_rope_T(kh) * (2.0 * g))
            kn = (kh * kh).sum(-1)
            w2 = (q[b] @ W_vo) * np.exp(-g * kn)[:, None]  # [S, 64]
            vsb_b.append(
                w2.reshape(16, 128, 64).transpose(1, 0, 2).reshape(128, 1024)
            )
            qn = (qh * qh).sum(-1)
            qn_exp[b, h] = np.exp(-g * qn)

        qro = np.concatenate(qro_b, 0).astype(BF16)  # [128, S]
        kro = np.concatenate(kro_b, 0).astype(BF16)
        vsb = np.concatenate(vsb_b, 1).astype(BF16)  # [128, 2*1024]

        in_maps.append(
            {
                "bmask": np.ascontiguousarray(bm),
                "qro": np.ascontiguousarray(qro),
                "kro": np.ascontiguousarray(kro),
                "vsb": np.ascontiguousarray(vsb),
            }
        )
    return in_maps, qn_exp


def kernel(q, Wq, Wk, Wv, Wo, gamma):
    global LAST_RESULTS
    from concourse import bass_utils

    nc = _get_program()
    in_maps, qn_exp = _prep_inputs(q, Wq, Wk, Wv, Wo, gamma)
    trace = bool(int(os.environ.get("KERNEL_TRACE", "0")))
    res = bass_utils.run_bass_kernel_spmd(
        nc, in_maps, core_ids=list(range(N_CORES)), trace=trace
    )
    LAST_RESULTS = res

    final = np.zeros((B, S, D), np.float32)
    for h in range(H):
        o = np.asarray(res.results[h]["out"], np.float32)  # [128, S]
        # col block i: [b0(64) | b1(64)] for s-strip i; row r = s offset
        o4 = o.reshape(128, 16, 2, 64)  # [r, i, b, e]
        for b in range(B):
            ob = o4[:, :, b, :].transpose(1, 0, 2).reshape(S, D)  # [s, e]
            final[b] += ob * qn_exp[b, h][:, None]
    return final
